# revision 1
# baseline (speedup 1.0000x reference)
"""Trainium2 Bass kernel for nn_CopiedSetEncoder (set encoder with recurrent
attention). Self-contained: shards batch across 8 NeuronCores, builds a
length-specialized SPMD Tile kernel, runs it, and reassembles the output.
"""
import os

import numpy as np

import concourse.bass as bass
import concourse.mybir as mybir
import concourse.tile as tile
from concourse.bass_utils import run_bass_kernel_spmd

B, F_, D_IN = 128, 1024, 128
H1, H2, E, H = 512, 512, 256, 256
N_SHUFFLE = 5
NCORES = 8
BLOC = B // NCORES  # 16 batches per core
NEG = -1e30
C1 = 15.0  # logit shift for max-free softmax

f32 = mybir.dt.float32
f16 = mybir.dt.float16


def _split_multi_waits(nc):
    """HW allows at most one sync wait per instruction; hoist extras into
    standalone InstEventSemaphore carriers on the same engine."""
    cnt = 0
    for bb in nc.main_func.blocks:
        insts = bb.instructions  # live list
        i = 0
        while i < len(insts):
            ins = insts[i]
            si = ins.sync_info
            if si is not None and si.on_wait and len(si.on_wait) > 1:
                waits = list(si.on_wait)
                carriers = []
                for w in waits[:-1]:
                    cnt += 1
                    ev = mybir.InstEventSemaphore(name=f"wsplit-{cnt}")
                    ev.engine = ins.engine
                    ev.sync_info = mybir.SyncInfo(on_wait=[w], on_update=[])
                    carriers.append(ev)
                ins.sync_info = mybir.SyncInfo(
                    on_wait=[waits[-1]], on_update=list(si.on_update)
                )
                for j, ev in enumerate(carriers):
                    insts.insert(i + j, ev)
                    nc.register_instruction(ev, overwrite=True)
                i += len(carriers)
            i += 1
    return cnt


PHASE = os.environ.get("KER_PHASE", "full")


def _build_module(n_chunks, t_common):
    """One SPMD program for all cores. n_chunks[j] = 128-token chunks for
    batch slot j (uniform across cores); t_common = sum(n_chunks)*128."""
    nc = bass.Bass()
    n_tiles = t_common // 512
    off = np.concatenate(([0], np.cumsum(np.asarray(n_chunks) * 128)))
    tot_chunks = t_common // 128
    max_nc = max(n_chunks)

    # ---- inputs ----
    xT_e = nc.declare_dram_parameter("xT", [128, t_common], f16, isOutput=False)
    w1_e = nc.declare_dram_parameter("w1", [128, H1], f16, isOutput=False)
    w2_e = nc.declare_dram_parameter("w2", [128, 4, H2], f16, isOutput=False)
    w3_e = nc.declare_dram_parameter("w3", [128, 4, E], f16, isOutput=False)
    wih_e = nc.declare_dram_parameter("wih", [128, 2, 4 * H], f16, isOutput=False)
    whh_e = nc.declare_dram_parameter("whh", [128, 2, 4 * H], f16, isOutput=False)
    b1_e = nc.declare_dram_parameter("b1", [128, 4], f32, isOutput=False)
    b2_e = nc.declare_dram_parameter("b2", [128, 4], f32, isOutput=False)
    bg_e = nc.declare_dram_parameter("bg", [128, 8], f32, isOutput=False)
    mask_e = nc.declare_dram_parameter(
        "mask", [128, tot_chunks, BLOC], f32, isOutput=False
    )
    w0T_e = nc.declare_dram_parameter(
        "w0T", [128, tot_chunks, BLOC], f16, isOutput=False
    )
    ones1_e = nc.declare_dram_parameter("ones1", [1, 128], f32, isOutput=False)
    ident_e = nc.declare_dram_parameter("ident", [128, 128], f32, isOutput=False)
    att_o = nc.declare_dram_parameter("att", [BLOC, E], f32, isOutput=True)
    qt_o = nc.declare_dram_parameter("qt", [BLOC, H], f32, isOutput=True)

    with tile.TileContext(nc) as tc:
        with tc.tile_pool(name="big", bufs=1) as big, \
             tc.tile_pool(name="wp", bufs=1) as wp:
            # resident tensors
            xT = big.tile([128, t_common], f16)
            embA = big.tile([128, 2, t_common], f16)
            embB = big.tile([128, tot_chunks, E], f16)
            w1 = wp.tile([128, H1], f16)
            w2 = wp.tile([128, 4, H2], f16)
            w3 = wp.tile([128, 4, E], f16)
            wih = wp.tile([128, 2, 4 * H], f16)
            whh = wp.tile([128, 2, 4 * H], f16)
            b1 = wp.tile([128, 4], f32)
            b2 = wp.tile([128, 4], f32)
            bg = wp.tile([128, 8], f32)
            mask = wp.tile([128, tot_chunks, BLOC], f32)
            w0T = wp.tile([128, tot_chunks, BLOC], f16)
            ones1 = wp.tile([1, 128], f32)
            ident = wp.tile([128, 128], f32)
            for dst, src in [
                (xT, xT_e), (w1, w1_e), (w2, w2_e), (w3, w3_e),
                (wih, wih_e), (whh, whh_e), (b1, b1_e), (b2, b2_e),
                (bg, bg_e), (mask, mask_e), (w0T, w0T_e), (ident, ident_e),
                (ones1, ones1_e),
            ]:
                nc.sync.dma_start(out=dst[:], in_=src[:])

            # ---- phase 1: MLP over 512-token tiles ----
            with tc.tile_pool(name="mlp", bufs=3) as mp, \
                 tc.tile_pool(name="ps1", bufs=2, space="PSUM") as ps1, \
                 tc.tile_pool(name="ps2", bufs=2, space="PSUM") as ps2, \
                 tc.tile_pool(name="ps3", bufs=2, space="PSUM") as ps3:
                for t in range(n_tiles):
                    sl = slice(t * 512, (t + 1) * 512)
                    h1t = mp.tile([128, 4, 512], f16, tag="h1")
                    for mc in range(4):
                        p = ps1.tile([128, 512], f32, tag="pA")
                        nc.tensor.matmul(
                            p[:], w1[:, mc * 128:(mc + 1) * 128], xT[:, sl],
                            start=True, stop=True,
                        )
                        if mc % 2 == 0:
                            nc.scalar.activation(
                                out=h1t[:, mc, :], in_=p[:],
                                func=mybir.ActivationFunctionType.Relu,
                                bias=b1[:, mc:mc + 1], scale=1.0,
                            )
                        else:
                            nc.vector.tensor_scalar(
                                out=h1t[:, mc, :], in0=p[:], scalar1=b1[:, mc:mc + 1],
                                scalar2=0.0, op0=mybir.AluOpType.add,
                                op1=mybir.AluOpType.max,
                            )
                    h2t = mp.tile([128, 4, 512], f16, tag="h2")
                    for mc in range(4):
                        p = ps2.tile([128, 512], f32, tag="pB")
                        for kc in range(4):
                            nc.tensor.matmul(
                                p[:], w2[:, kc, mc * 128:(mc + 1) * 128],
                                h1t[:, kc, :], start=(kc == 0), stop=(kc == 3),
                            )
                        if mc % 2 == 0:
                            nc.scalar.activation(
                                out=h2t[:, mc, :], in_=p[:],
                                func=mybir.ActivationFunctionType.Relu,
                                bias=b2[:, mc:mc + 1], scale=1.0,
                            )
                        else:
                            nc.vector.tensor_scalar(
                                out=h2t[:, mc, :], in0=p[:], scalar1=b2[:, mc:mc + 1],
                                scalar2=0.0, op0=mybir.AluOpType.add,
                                op1=mybir.AluOpType.max,
                            )
                    # embA: [e-chunk partitions, tokens]
                    for mc in range(2):
                        p = ps3.tile([128, 512], f32, tag="pC")
                        for kc in range(4):
                            nc.tensor.matmul(
                                p[:], w3[:, kc, mc * 128:(mc + 1) * 128],
                                h2t[:, kc, :], start=(kc == 0), stop=(kc == 3),
                            )
                        nc.scalar.copy(out=embA[:, mc, sl], in_=p[:])
                    # embB: [token partitions, e] via h2-stationary matmuls
                    for s in range(4):
                        p = ps3.tile([128, 256], f32, tag="pD")
                        tsl = slice(s * 128, (s + 1) * 128)
                        for kc in range(4):
                            nc.tensor.matmul(
                                p[:], h2t[:, kc, tsl], w3[:, kc, :],
                                start=(kc == 0), stop=(kc == 3),
                            )
                        nc.vector.tensor_copy(embB[:, t * 4 + s, :], p[:])

            if PHASE == "mlp":
                with tc.tile_pool(name="dummy", bufs=1) as dp:
                    da = dp.tile([BLOC, E], f32)
                    dq = dp.tile([BLOC, H], f32)
                    nc.vector.tensor_copy(da[:], embA[:BLOC, 0, :E])
                    nc.vector.tensor_copy(dq[:], embB[:BLOC, 0, :])
                    nc.sync.dma_start(out=att_o[:], in_=da[:])
                    nc.sync.dma_start(out=qt_o[:], in_=dq[:])

            # ---- phase 2: recurrent attention ----
            if PHASE != "mlp":
              with tc.tile_pool(name="att", bufs=1) as ap, \
                 tc.tile_pool(name="attd", bufs=2) as ad, \
                 tc.tile_pool(name="psL", bufs=2, space="PSUM") as psL, \
                 tc.tile_pool(name="psA", bufs=1, space="PSUM") as psA, \
                 tc.tile_pool(name="psG", bufs=1, space="PSUM") as psG, \
                 tc.tile_pool(name="psT", bufs=1, space="PSUM") as psT:
                qtT = ap.tile([128, 2, BLOC], f16)      # query, [h, b]
                qtT32 = ap.tile([128, 2, BLOC], f32)
                ct = ap.tile([128, 2, BLOC], f32)       # cell state
                att_sb = ap.tile([BLOC, E], f32)
                attT = ap.tile([128, 2, BLOC], f16)
                lgT = ap.tile([128, tot_chunks, BLOC], f32)  # shifted logits
                onesc = ap.tile([128, 1], f32)
                wTn = ap.tile([128, tot_chunks, BLOC], f16)  # normalized weights
                nc.vector.memset(qtT[:], 0.0)
                nc.vector.memset(ct[:], 0.0)
                nc.vector.memset(onesc[:], 1.0)

                n_grp = (tot_chunks + 7) // 8
                for it in range(N_SHUFFLE):
                    if it > 0:
                        # logits token-major: chunk-stationary matmuls
                        for g in range(n_grp):
                            nch = min(8, tot_chunks - g * 8)
                            lgp = psL.tile([128, 8, BLOC], f32, tag="lgp")
                            for ci in range(nch):
                                c = g * 8 + ci
                                for kc in range(2):
                                    nc.tensor.matmul(
                                        lgp[:, ci, :],
                                        embA[:, kc, c * 128:(c + 1) * 128],
                                        qtT[:, kc, :],
                                        start=(kc == 0), stop=(kc == 1),
                                    )
                            # shift+mask into sbuf (f32)
                            nc.vector.tensor_tensor(
                                out=lgT[:, g * 8: g * 8 + nch, :],
                                in0=lgp[:, :nch, :],
                                in1=mask[:, g * 8: g * 8 + nch, :],
                                op=mybir.AluOpType.add,
                            )
                        # exp (pass 1, fp32, pre-shifted by mask)
                        w1T = ad.tile([128, tot_chunks, BLOC], f32, tag="w1T")
                        nc.scalar.activation(
                            out=w1T[:], in_=lgT[:],
                            func=mybir.ActivationFunctionType.Exp,
                        )
                        # S_j = sum over tokens: fp32 chunk matmuls vs ones
                        s_ps = psT.tile([BLOC, 1], f32, tag="t1")
                        for c in range(tot_chunks):
                            nc.tensor.matmul(
                                s_ps[:, :], w1T[:, c, :], onesc[:],
                                start=(c == 0), stop=(c == tot_chunks - 1),
                            )
                        rS = ad.tile([BLOC, 1], f32, tag="rS")
                        nc.vector.reciprocal(rS[:], s_ps[:])
                        # broadcast 1/S to [128, BLOC]: transpose + K=1 matmul
                        rT_ps = psT.tile([1, BLOC], f32, tag="t1")
                        nc.tensor.transpose(rT_ps[:], rS[:], ident[:BLOC, :BLOC])
                        rRow = ad.tile([1, BLOC], f32, tag="rRow")
                        nc.vector.tensor_copy(rRow[:], rT_ps[:])
                        rB_ps = psT.tile([128, BLOC], f32, tag="t1")
                        nc.tensor.matmul(
                            rB_ps[:], ones1[:], rRow[:], start=True, stop=True
                        )
                        rB = ad.tile([128, BLOC], f32, tag="rB2")
                        nc.vector.tensor_copy(rB[:], rB_ps[:])
                        # normalized fp16 weights: w1T * (1/S) broadcast
                        rb_ap = rB[:]
                        rB_b = bass.AP(
                            tensor=rb_ap.tensor, offset=rb_ap.offset,
                            ap=[list(rb_ap.ap[0]), [0, tot_chunks],
                                list(rb_ap.ap[1])],
                        )
                        nc.vector.tensor_tensor(
                            out=wTn[:], in0=w1T[:], in1=rB_b,
                            op=mybir.AluOpType.mult,
                        )
                        wsrc = wTn
                    else:
                        wsrc = w0T

                    # attended: accumulate all chunks, M=16
                    att_ps = psA.tile([BLOC, E], f32)
                    for c in range(tot_chunks):
                        nc.tensor.matmul(
                            att_ps[:, :], wsrc[:, c, :], embB[:, c, :],
                            start=(c == 0), stop=(c == tot_chunks - 1),
                        )
                    nc.vector.tensor_copy(att_sb[:], att_ps[:])
                    # attT: [16, 256] -> [128, 2, 16]
                    for c in range(2):
                        pt = psT.tile([128, BLOC], f32, tag="pt")
                        nc.tensor.transpose(
                            pt[:], att_sb[:, c * 128:(c + 1) * 128],
                            ident[:BLOC, :BLOC],
                        )
                        nc.scalar.copy(out=attT[:, c, :], in_=pt[:])

                    # LSTM gates = Wih @ att + Whh @ qt + bg
                    g_ps = psG.tile([128, 8, BLOC], f32)
                    for mc in range(8):
                        msl = slice(mc * 128, (mc + 1) * 128)
                        for kc in range(2):
                            nc.tensor.matmul(
                                g_ps[:, mc, :], wih[:, kc, msl], attT[:, kc, :],
                                start=(kc == 0), stop=False,
                            )
                        for kc in range(2):
                            nc.tensor.matmul(
                                g_ps[:, mc, :], whh[:, kc, msl],
                                qtT[:, kc, :],
                                start=False, stop=(kc == 1),
                            )
                    ig = ad.tile([128, 2, BLOC], f32, tag="ig")
                    fg = ad.tile([128, 2, BLOC], f32, tag="fg")
                    gg = ad.tile([128, 2, BLOC], f32, tag="gg")
                    og = ad.tile([128, 2, BLOC], f32, tag="og")
                    for c in range(2):
                        nc.scalar.activation(
                            out=ig[:, c, :], in_=g_ps[:, c, :],
                            func=mybir.ActivationFunctionType.Sigmoid,
                            bias=bg[:, c:c + 1], scale=1.0,
                        )
                        nc.scalar.activation(
                            out=fg[:, c, :], in_=g_ps[:, 2 + c, :],
                            func=mybir.ActivationFunctionType.Sigmoid,
                            bias=bg[:, 2 + c:3 + c], scale=1.0,
                        )
                        nc.scalar.activation(
                            out=gg[:, c, :], in_=g_ps[:, 4 + c, :],
                            func=mybir.ActivationFunctionType.Tanh,
                            bias=bg[:, 4 + c:5 + c], scale=1.0,
                        )
                        nc.scalar.activation(
                            out=og[:, c, :], in_=g_ps[:, 6 + c, :],
                            func=mybir.ActivationFunctionType.Sigmoid,
                            bias=bg[:, 6 + c:7 + c], scale=1.0,
                        )
                    tmp = ad.tile([128, 2, BLOC], f32, tag="tmp")
                    nc.vector.tensor_tensor(
                        out=tmp[:], in0=ig[:], in1=gg[:], op=mybir.AluOpType.mult
                    )
                    nc.vector.tensor_tensor(
                        out=ct[:], in0=fg[:], in1=ct[:], op=mybir.AluOpType.mult
                    )
                    nc.vector.tensor_tensor(
                        out=ct[:], in0=ct[:], in1=tmp[:], op=mybir.AluOpType.add
                    )
                    th = ad.tile([128, 2, BLOC], f32, tag="th")
                    for c in range(2):
                        nc.scalar.activation(
                            out=th[:, c, :], in_=ct[:, c, :],
                            func=mybir.ActivationFunctionType.Tanh,
                        )
                    nc.vector.tensor_tensor(
                        out=qtT32[:], in0=og[:], in1=th[:], op=mybir.AluOpType.mult
                    )
                    nc.vector.tensor_copy(qtT[:], qtT32[:])

                # outputs
                nc.sync.dma_start(out=att_o[:], in_=att_sb[:])
                qt_out = ap.tile([BLOC, H], f32)
                for c in range(2):
                    pt = psT.tile([BLOC, 128], f32, tag="ptq")
                    nc.tensor.transpose(pt[:], qtT32[:, c, :], ident[:, :])
                    nc.vector.tensor_copy(qt_out[:, c * 128:(c + 1) * 128], pt[:])
                nc.sync.dma_start(out=qt_o[:], in_=qt_out[:])

    _split_multi_waits(nc)
    return nc


def kernel(state, length, W1, b1, W2, b2, W3, b3, W_ih, W_hh, b_ih, b_hh):
    state = np.asarray(state, dtype=np.float32)
    length = np.asarray(length, dtype=np.int32)
    lengths = length.astype(np.int64)

    # sorted snake assignment: slot j of core c = order[j*8 + c]
    order = np.argsort(-lengths, kind="stable")
    perm = order.reshape(BLOC, NCORES)  # perm[j, c]
    lens = lengths[perm]  # [BLOC, NCORES]
    n_chunks = [int(max(1, int(np.max(np.ceil(lens[j] / 128))))) for j in range(BLOC)]
    t_common = -(-(sum(n_chunks) * 128) // 512) * 512  # pad to 512 multiple
    off = np.concatenate(([0], np.cumsum(np.asarray(n_chunks) * 128)))
    tot_chunks = t_common // 128

    nc = _build_module(n_chunks, t_common)

    # host-side weight prep (shared across cores)
    w1h = W1.T.astype(np.float16)                                # [128, 512]
    w2h = np.ascontiguousarray(
        W2.T.reshape(4, 128, H2).transpose(1, 0, 2)
    ).astype(np.float16)                                         # [128, 4, 512]
    w3h = np.ascontiguousarray(
        W3.T.reshape(4, 128, E).transpose(1, 0, 2)
    ).astype(np.float16)                                         # [128, 4, 256]
    wihh = np.ascontiguousarray(
        W_ih.T.reshape(2, 128, 4 * H).transpose(1, 0, 2)
    ).astype(np.float16)                                         # [128, 2, 1024]
    whhh = np.ascontiguousarray(
        W_hh.T.reshape(2, 128, 4 * H).transpose(1, 0, 2)
    ).astype(np.float16)
    b1h = np.ascontiguousarray(b1.reshape(4, 128).T).astype(np.float32)
    b2h = np.ascontiguousarray(b2.reshape(4, 128).T).astype(np.float32)
    bgv = (b_ih + b_hh + W_ih @ b3).astype(np.float32)
    bgh = np.ascontiguousarray(bgv.reshape(8, 128).T).astype(np.float32)
    identh = np.eye(128, dtype=np.float32)
    ones1h = np.ones((1, 128), dtype=np.float32)

    in_maps = []
    for c in range(NCORES):
        bidx = perm[:, c]  # batch index per slot
        xT = np.zeros((128, t_common), dtype=np.float16)
        maskh = np.full((128, tot_chunks, BLOC), NEG, dtype=np.float32)
        w0T = np.zeros((128, tot_chunks, BLOC), dtype=np.float16)
        for j in range(BLOC):
            ln = int(lengths[bidx[j]])
            xT[:, off[j]: off[j] + ln] = state[bidx[j], :ln, :].T
            c0 = off[j] // 128
            valid = np.zeros(n_chunks[j] * 128, dtype=bool)
            valid[:ln] = True
            vT = valid.reshape(n_chunks[j], 128).T  # [128, n_chunks_j]
            maskh[:, c0:c0 + n_chunks[j], j] = np.where(vT, -C1, NEG)
            w0T[:, c0:c0 + n_chunks[j], j] = np.where(
                vT, 1.0 / ln, 0.0
            ).astype(np.float16)
        in_maps.append({
            "xT": xT, "w1": w1h, "w2": w2h, "w3": w3h,
            "wih": wihh, "whh": whhh, "b1": b1h, "b2": b2h, "bg": bgh,
            "mask": maskh, "w0T": w0T, "ident": identh, "ones1": ones1h,
        })

    res = run_bass_kernel_spmd(nc, in_maps, list(range(NCORES)))

    out = np.zeros((B, E + H), dtype=np.float32)
    for c in range(NCORES):
        att = res.results[c]["att"] + b3[None, :].astype(np.float32)
        qt = res.results[c]["qt"]
        for j in range(BLOC):
            out[perm[j, c], :E] = att[j]
            out[perm[j, c], E:] = qt[j]
    return out



# revision 5
# speedup vs baseline: 1.6582x; 1.6582x over previous
"""Trainium2 Bass kernel for nn_CopiedSetEncoder (set encoder with recurrent
attention). Self-contained: shards batch across 8 NeuronCores with a balanced
contiguous token packing, builds a length-specialized SPMD Tile kernel in
bf16, runs it, and reassembles the output.

Structure per core (16 sequences packed into T tokens):
  phase 1  MLP over 512-token tiles -> embA [E-major] and embB [token-major]
           (embB via PE transposes of embA, not extra matmuls)
  phase 2  5 recurrent-attention iterations; softmax uses unnormalized
           bf16 exp weights, per-group DVE reductions for the sums, and
           column-tiled (4x concurrent) attended matmuls; LSTM activations
           are computed from Exp only (no activation-table swaps).
"""
import numpy as np
import ml_dtypes

import concourse.bass as bass
import concourse.mybir as mybir
import concourse.tile as tile
from concourse.bass_utils import run_bass_kernel_spmd

B, F_, D_IN = 128, 1024, 128
H1, H2, E, H = 512, 512, 256, 256
N_SHUFFLE = 5
NCORES = 8
BLOC = B // NCORES  # 16 sequences per core
NEG = -1e30

f32 = mybir.dt.float32
bf16 = mybir.dt.bfloat16
BF = ml_dtypes.bfloat16
AF = mybir.ActivationFunctionType
OP = mybir.AluOpType


def _split_multi_waits(nc):
    """HW allows at most one sync wait per instruction; hoist extras into
    standalone InstEventSemaphore carriers on the same engine."""
    cnt = 0
    for bb in nc.main_func.blocks:
        insts = bb.instructions  # live list
        i = 0
        while i < len(insts):
            ins = insts[i]
            si = ins.sync_info
            if si is not None and si.on_wait and len(si.on_wait) > 1:
                waits = list(si.on_wait)
                carriers = []
                for w in waits[:-1]:
                    cnt += 1
                    ev = mybir.InstEventSemaphore(name=f"wsplit-{cnt}")
                    ev.engine = ins.engine
                    ev.sync_info = mybir.SyncInfo(on_wait=[w], on_update=[])
                    carriers.append(ev)
                ins.sync_info = mybir.SyncInfo(
                    on_wait=[waits[-1]], on_update=list(si.on_update)
                )
                for j, ev in enumerate(carriers):
                    insts.insert(i + j, ev)
                    nc.register_instruction(ev, overwrite=True)
                i += len(carriers)
            i += 1
    return cnt


def _view(t_ap, offset_elems, dims):
    """Build a strided free-dim view of a tile AP. dims = [[stride, size], ...]
    for the free dims; partition dim copied from the tile."""
    return bass.AP(
        tensor=t_ap.tensor,
        offset=t_ap.offset + offset_elems,
        ap=[list(t_ap.ap[0])] + [list(d) for d in dims],
    )


def _build_module(T):
    C = T // 128          # 128-token chunks
    NT = T // 512         # MLP tiles
    NG = (C + 7) // 8     # softmax groups of up to 8 chunks
    NQ = C // 4           # attended quads (C is a multiple of 4)

    nc = bass.Bass()

    # ---- inputs ----
    xT_e = nc.declare_dram_parameter("xT", [128, T], bf16, isOutput=False)
    w1_e = nc.declare_dram_parameter("w1", [128, H1], bf16, isOutput=False)
    w2_e = nc.declare_dram_parameter("w2", [128, 4, H2], bf16, isOutput=False)
    w3_e = nc.declare_dram_parameter("w3", [128, 4, E], bf16, isOutput=False)
    wih_e = nc.declare_dram_parameter("wih", [128, 2, 4 * H], bf16, isOutput=False)
    whh_e = nc.declare_dram_parameter("whh", [128, 2, 4 * H], bf16, isOutput=False)
    b1_e = nc.declare_dram_parameter("b1", [128, 4], f32, isOutput=False)
    b2_e = nc.declare_dram_parameter("b2", [128, 4], f32, isOutput=False)
    bg_e = nc.declare_dram_parameter("bg", [128, 8], f32, isOutput=False)
    mask_e = nc.declare_dram_parameter("mask", [128, C, BLOC], f32, isOutput=False)
    w0T_e = nc.declare_dram_parameter("w0T", [128, C, BLOC], bf16, isOutput=False)
    sel_e = nc.declare_dram_parameter("sel", [128, BLOC], bf16, isOutput=False)
    ident_e = nc.declare_dram_parameter("ident", [128, 128], bf16, isOutput=False)
    onesc_e = nc.declare_dram_parameter("onesc", [128, 1], f32, isOutput=False)
    att_o = nc.declare_dram_parameter("att", [BLOC, E], f32, isOutput=True)
    qt_o = nc.declare_dram_parameter("qt", [128, 2, BLOC], f32, isOutput=True)

    with tile.TileContext(nc) as tc:
        with tc.tile_pool(name="big", bufs=1) as big, \
             tc.tile_pool(name="wp", bufs=1) as wp:
            xT = big.tile([128, T], bf16)
            embA = big.tile([128, 2, T], bf16)
            embB = big.tile([128, C, E], bf16)
            w1T = big.tile([128, C, BLOC], bf16)
            Spart = big.tile([128, NG, BLOC], f32)
            w1 = wp.tile([128, H1], bf16)
            w2 = wp.tile([128, 4, H2], bf16)
            w3 = wp.tile([128, 4, E], bf16)
            wih = wp.tile([128, 2, 4 * H], bf16)
            whh = wp.tile([128, 2, 4 * H], bf16)
            b1 = wp.tile([128, 4], f32)
            b2 = wp.tile([128, 4], f32)
            bg = wp.tile([128, 8], f32)
            mask = wp.tile([128, C, BLOC], f32)
            w0T = wp.tile([128, C, BLOC], bf16)
            sel = wp.tile([128, BLOC], bf16)
            ident = wp.tile([128, 128], bf16)
            onesc = wp.tile([128, 1], f32)

            # weight DMAs needed by the first MLP tile go first; xT is
            # DMA'd per tile inside the loop; attention-only inputs later.
            for dst, src in [(w1, w1_e), (b1, b1_e), (w2, w2_e), (b2, b2_e),
                             (w3, w3_e), (ident, ident_e)]:
                nc.sync.dma_start(out=dst[:], in_=src[:])

            # ---- phase 1: MLP over 512-token tiles ----
            with tc.tile_pool(name="mlp", bufs=3) as mp, \
                 tc.tile_pool(name="ps1", bufs=2, space="PSUM") as ps1, \
                 tc.tile_pool(name="ps2", bufs=2, space="PSUM") as ps2, \
                 tc.tile_pool(name="ps3", bufs=2, space="PSUM") as ps3, \
                 tc.tile_pool(name="psE", bufs=2, space="PSUM") as psE:
                for t in range(NT):
                    sl = slice(t * 512, (t + 1) * 512)
                    nc.sync.dma_start(out=xT[:, sl], in_=xT_e[:, sl])
                    if t == 1:
                        # attention-phase inputs, overlapped with compute
                        for dst, src in [(mask, mask_e), (w0T, w0T_e),
                                         (wih, wih_e), (whh, whh_e),
                                         (bg, bg_e), (sel, sel_e),
                                         (onesc, onesc_e)]:
                            nc.sync.dma_start(out=dst[:], in_=src[:])
                    h1t = mp.tile([128, 4, 512], bf16, tag="h1")
                    for mc in range(4):
                        p = ps1.tile([128, 512], f32, tag="pA")
                        nc.tensor.matmul(
                            p[:], w1[:, mc * 128:(mc + 1) * 128], xT[:, sl],
                            start=True, stop=True,
                        )
                        if mc % 2 == 0:
                            nc.scalar.activation(
                                out=h1t[:, mc, :], in_=p[:], func=AF.Relu,
                                bias=b1[:, mc:mc + 1], scale=1.0,
                            )
                        else:
                            nc.vector.tensor_scalar(
                                out=h1t[:, mc, :], in0=p[:],
                                scalar1=b1[:, mc:mc + 1], scalar2=0.0,
                                op0=OP.add, op1=OP.max,
                            )
                    h2t = mp.tile([128, 4, 512], bf16, tag="h2")
                    for mc in range(4):
                        p = ps2.tile([128, 512], f32, tag="pB")
                        for kc in range(4):
                            nc.tensor.matmul(
                                p[:], w2[:, kc, mc * 128:(mc + 1) * 128],
                                h1t[:, kc, :], start=(kc == 0), stop=(kc == 3),
                            )
                        if mc % 2 == 0:
                            nc.scalar.activation(
                                out=h2t[:, mc, :], in_=p[:], func=AF.Relu,
                                bias=b2[:, mc:mc + 1], scale=1.0,
                            )
                        else:
                            nc.vector.tensor_scalar(
                                out=h2t[:, mc, :], in0=p[:],
                                scalar1=b2[:, mc:mc + 1], scalar2=0.0,
                                op0=OP.add, op1=OP.max,
                            )
                    for mc in range(2):
                        p = ps3.tile([128, 512], f32, tag="pC")
                        for kc in range(4):
                            nc.tensor.matmul(
                                p[:], w3[:, kc, mc * 128:(mc + 1) * 128],
                                h2t[:, kc, :], start=(kc == 0), stop=(kc == 3),
                            )
                        nc.scalar.copy(out=embA[:, mc, sl], in_=p[:])
                    # embB (token-major) via PE transposes of embA
                    for s in range(4):
                        for kc in range(2):
                            pt = psE.tile([128, 128], bf16, tag="pT")
                            nc.tensor.transpose(
                                pt[:],
                                embA[:, kc, t * 512 + s * 128:
                                     t * 512 + (s + 1) * 128],
                                ident[:, :],
                            )
                            nc.vector.tensor_copy(
                                embB[:, t * 4 + s, kc * 128:(kc + 1) * 128],
                                pt[:],
                            )

            # ---- phase 2: recurrent attention ----
            with tc.tile_pool(name="att", bufs=1) as ap, \
                 tc.tile_pool(name="attd", bufs=2) as ad, \
                 tc.tile_pool(name="psL", bufs=2, space="PSUM") as psL, \
                 tc.tile_pool(name="psA", bufs=1, space="PSUM") as psA, \
                 tc.tile_pool(name="psG", bufs=1, space="PSUM") as psG, \
                 tc.tile_pool(name="psC", bufs=1, space="PSUM") as psC, \
                 tc.tile_pool(name="psT", bufs=1, space="PSUM") as psT:
                qtT = ap.tile([128, 2, BLOC], bf16)
                qtT32 = ap.tile([128, 2, BLOC], f32)
                ct = ap.tile([128, 2, BLOC], f32)
                att_bf = ap.tile([BLOC, E], bf16)
                att_f = ap.tile([BLOC, E], f32)
                attT = ap.tile([128, 2, BLOC], bf16)
                S_sb = ap.tile([128, BLOC], f32)
                rS = ap.tile([BLOC, 1], f32)
                att_ps = psA.tile([128, E], f32)
                nc.vector.memset(qtT[:], 0.0)
                nc.vector.memset(att_ps[:], 0.0)  # dead rows stay 0 forever

                w1ap = w1T[:]
                spap = Spart[:]

                def emit_att_quad(q, wsrc):
                    for j in range(4):
                        c = 4 * q + j
                        nc.tensor.matmul(
                            att_ps[32 * j:32 * j + BLOC, :],
                            wsrc[:, c, :], embB[:, c, :],
                            start=(q == 0), stop=(q == NQ - 1),
                            tile_position=(0, 32 * j), skip_group_check=True,
                        )

                for it in range(N_SHUFFLE):
                    if it > 0:
                        # logits (chunk-stationary), masked exp per group,
                        # attended quads of group g-1 interleaved
                        for g in range(NG):
                            nch = min(8, C - 8 * g)
                            lgp = psL.tile([128, 8, BLOC], f32, tag="lgp")
                            for ci in range(nch):
                                c = 8 * g + ci
                                for kc in range(2):
                                    nc.tensor.matmul(
                                        lgp[:, ci, :],
                                        embA[:, kc, c * 128:(c + 1) * 128],
                                        qtT[:, kc, :],
                                        start=(kc == 0), stop=(kc == 1),
                                    )
                            lgm = ad.tile([128, 8, BLOC], f32, tag="lgm")
                            nc.vector.tensor_tensor(
                                out=lgm[:, :nch, :], in0=lgp[:, :nch, :],
                                in1=mask[:, 8 * g:8 * g + nch, :], op=OP.add,
                            )
                            nc.scalar.activation(
                                out=w1T[:, 8 * g:8 * g + nch, :],
                                in_=lgm[:, :nch, :], func=AF.Exp,
                            )
                            gview = _view(w1ap, 8 * g * BLOC,
                                          [[1, BLOC], [BLOC, nch]])
                            nc.vector.tensor_reduce(
                                out=Spart[:, g, :], in_=gview,
                                axis=mybir.AxisListType.X, op=OP.add,
                            )
                            for qq in range(2 * g - 2, 2 * g):
                                if 0 <= qq < NQ:
                                    emit_att_quad(qq, w1T)
                        for qq in range(2 * NG - 2, NQ):
                            emit_att_quad(qq, w1T)
                        # S and 1/S
                        sview = _view(spap, 0, [[1, BLOC], [BLOC, NG]])
                        nc.vector.tensor_reduce(
                            out=S_sb[:], in_=sview,
                            axis=mybir.AxisListType.X, op=OP.add,
                        )
                        s_ps = psT.tile([BLOC, 1], f32, tag="sps")
                        nc.tensor.matmul(s_ps[:], S_sb[:], onesc[:],
                                         start=True, stop=True)
                        nc.vector.reciprocal(rS[:], s_ps[:])
                    else:
                        for qq in range(NQ):
                            emit_att_quad(qq, w0T)

                    # combine 4 column partials via selector matmul
                    attC = ad.tile([128, E], bf16, tag="attC")
                    nc.vector.tensor_copy(attC[:], att_ps[:])
                    comb = psC.tile([BLOC, E], f32, tag="comb")
                    nc.tensor.matmul(comb[:], sel[:], attC[:],
                                     start=True, stop=True)
                    if it == 0:
                        nc.vector.tensor_copy(att_bf[:], comb[:])
                    else:
                        nc.vector.tensor_scalar(
                            out=att_bf[:], in0=comb[:], scalar1=rS[:],
                            scalar2=0.0, op0=OP.mult, op1=OP.add,
                        )
                        if it == N_SHUFFLE - 1:
                            nc.vector.tensor_scalar(
                                out=att_f[:], in0=comb[:], scalar1=rS[:],
                                scalar2=0.0, op0=OP.mult, op1=OP.add,
                            )
                    for c2 in range(2):
                        pt = psT.tile([128, BLOC], bf16, tag="ptA")
                        nc.tensor.transpose(
                            pt[:], att_bf[:, c2 * 128:(c2 + 1) * 128],
                            ident[:BLOC, :BLOC],
                        )
                        nc.scalar.copy(out=attT[:, c2, :], in_=pt[:])

                    # LSTM gates; gate chunk order [i0 i1 f0 f1 o0 o1 g0 g1]
                    g_ps = psG.tile([128, 8, BLOC], f32, tag="g")
                    for mc in range(8):
                        msl = slice(mc * 128, (mc + 1) * 128)
                        if it == 0:
                            for kc in range(2):
                                nc.tensor.matmul(
                                    g_ps[:, mc, :], wih[:, kc, msl],
                                    attT[:, kc, :],
                                    start=(kc == 0), stop=(kc == 1),
                                )
                        else:
                            for kc in range(2):
                                nc.tensor.matmul(
                                    g_ps[:, mc, :], wih[:, kc, msl],
                                    attT[:, kc, :],
                                    start=(kc == 0), stop=False,
                                )
                            for kc in range(2):
                                nc.tensor.matmul(
                                    g_ps[:, mc, :], whh[:, kc, msl],
                                    qtT[:, kc, :],
                                    start=False, stop=(kc == 1),
                                )
                    bgv = _view(bg[:], 0, [[1, 8], [0, BLOC]])
                    gb = ad.tile([128, 8, BLOC], f32, tag="gb")
                    nc.vector.tensor_tensor(out=gb[:], in0=g_ps[:], in1=bgv,
                                            op=OP.add)
                    # sigmoid(x) = 1/(1+exp(-x)) ; tanh(x) = 2/(1+exp(-2x))-1
                    e6 = ad.tile([128, 6, BLOC], f32, tag="e6")
                    nc.scalar.activation(out=e6[:], in_=gb[:, 0:6, :],
                                         func=AF.Exp, scale=-1.0)
                    t6 = ad.tile([128, 6, BLOC], f32, tag="t6")
                    nc.vector.tensor_scalar(out=t6[:], in0=e6[:], scalar1=1.0,
                                            scalar2=0.0, op0=OP.add, op1=OP.add)
                    sg6 = ad.tile([128, 6, BLOC], f32, tag="sg6")
                    nc.vector.reciprocal(sg6[:], t6[:])
                    e2 = ad.tile([128, 2, BLOC], f32, tag="e2")
                    nc.scalar.activation(out=e2[:], in_=gb[:, 6:8, :],
                                         func=AF.Exp, scale=-2.0)
                    t2 = ad.tile([128, 2, BLOC], f32, tag="t2")
                    nc.vector.tensor_scalar(out=t2[:], in0=e2[:], scalar1=1.0,
                                            scalar2=0.0, op0=OP.add, op1=OP.add)
                    r2 = ad.tile([128, 2, BLOC], f32, tag="r2")
                    nc.vector.reciprocal(r2[:], t2[:])
                    th2 = ad.tile([128, 2, BLOC], f32, tag="th2")
                    nc.vector.tensor_scalar(out=th2[:], in0=r2[:], scalar1=2.0,
                                            scalar2=-1.0, op0=OP.mult,
                                            op1=OP.add)
                    tmp = ad.tile([128, 2, BLOC], f32, tag="tmp")
                    nc.vector.tensor_tensor(out=tmp[:], in0=sg6[:, 0:2, :],
                                            in1=th2[:], op=OP.mult)
                    if it == 0:
                        nc.vector.tensor_copy(ct[:], tmp[:])
                    else:
                        nc.vector.tensor_tensor(out=ct[:], in0=sg6[:, 2:4, :],
                                                in1=ct[:], op=OP.mult)
                        nc.vector.tensor_tensor(out=ct[:], in0=ct[:],
                                                in1=tmp[:], op=OP.add)
                    ec = ad.tile([128, 2, BLOC], f32, tag="ec")
                    nc.scalar.activation(out=ec[:], in_=ct[:], func=AF.Exp,
                                         scale=-2.0)
                    tc2 = ad.tile([128, 2, BLOC], f32, tag="tc2")
                    nc.vector.tensor_scalar(out=tc2[:], in0=ec[:], scalar1=1.0,
                                            scalar2=0.0, op0=OP.add, op1=OP.add)
                    rc2 = ad.tile([128, 2, BLOC], f32, tag="rc2")
                    nc.vector.reciprocal(rc2[:], tc2[:])
                    thc = ad.tile([128, 2, BLOC], f32, tag="thc")
                    nc.vector.tensor_scalar(out=thc[:], in0=rc2[:], scalar1=2.0,
                                            scalar2=-1.0, op0=OP.mult,
                                            op1=OP.add)
                    nc.vector.tensor_tensor(out=qtT32[:], in0=sg6[:, 4:6, :],
                                            in1=thc[:], op=OP.mult)
                    nc.vector.tensor_copy(qtT[:], qtT32[:])

                nc.sync.dma_start(out=att_o[:], in_=att_f[:])
                nc.sync.dma_start(out=qt_o[:], in_=qtT32[:])

    _split_multi_waits(nc)
    return nc


def kernel(state, length, W1, b1, W2, b2, W3, b3, W_ih, W_hh, b_ih, b_hh):
    state = np.asarray(state, dtype=np.float32)
    lengths = np.asarray(length).astype(np.int64)

    # balanced assignment: greedy longest-first onto least-loaded core
    # that still has a free slot (16 per core)
    order = np.argsort(-lengths, kind="stable")
    core_slots = [[] for _ in range(NCORES)]
    core_sum = np.zeros(NCORES, dtype=np.int64)
    for idx in order:
        free = [c for c in range(NCORES) if len(core_slots[c]) < BLOC]
        c = min(free, key=lambda c: core_sum[c])
        core_slots[c].append(int(idx))
        core_sum[c] += lengths[idx]
    T = int(-(-int(core_sum.max()) // 512) * 512)
    C = T // 128

    nc = _build_module(T)

    # shared weights, bf16, matmul-ready layouts
    w1h = np.ascontiguousarray(W1.T).astype(BF)
    w2h = np.ascontiguousarray(
        W2.T.reshape(4, 128, H2).transpose(1, 0, 2)).astype(BF)
    w3h = np.ascontiguousarray(
        W3.T.reshape(4, 128, E).transpose(1, 0, 2)).astype(BF)
    # gate-chunk reorder [i f g o] -> [i f o g] so sigmoid gates are contiguous
    ridx = np.r_[0:512, 768:1024, 512:768]
    wihh = np.ascontiguousarray(
        W_ih[ridx].T.reshape(2, 128, 4 * H).transpose(1, 0, 2)).astype(BF)
    whhh = np.ascontiguousarray(
        W_hh[ridx].T.reshape(2, 128, 4 * H).transpose(1, 0, 2)).astype(BF)
    bgv = (b_ih + b_hh + W_ih @ b3)[ridx].astype(np.float32)
    bgh = np.ascontiguousarray(bgv.reshape(8, 128).T).astype(np.float32)
    b1h = np.ascontiguousarray(b1.reshape(4, 128).T).astype(np.float32)
    b2h = np.ascontiguousarray(b2.reshape(4, 128).T).astype(np.float32)
    selh = np.zeros((128, BLOC), dtype=BF)
    for k in range(4):
        for j in range(BLOC):
            selh[32 * k + j, j] = 1.0
    identh = np.eye(128, dtype=BF)
    onesh = np.ones((128, 1), dtype=np.float32)

    in_maps = []
    for c in range(NCORES):
        xT = np.zeros((128, T), dtype=BF)
        maskh = np.full((128, C, BLOC), NEG, dtype=np.float32)
        w0T = np.zeros((128, C, BLOC), dtype=BF)
        off = 0
        for j, seq in enumerate(core_slots[c]):
            ln = int(lengths[seq])
            xT[:, off:off + ln] = state[seq, :ln, :].T.astype(BF)
            tt = np.arange(off, off + ln)
            maskh[tt % 128, tt // 128, j] = 0.0
            w0T[tt % 128, tt // 128, j] = BF(1.0 / ln)
            off += ln
        in_maps.append({
            "xT": xT, "w1": w1h, "w2": w2h, "w3": w3h,
            "wih": wihh, "whh": whhh, "b1": b1h, "b2": b2h, "bg": bgh,
            "mask": maskh, "w0T": w0T, "sel": selh, "ident": identh,
            "onesc": onesh,
        })

    res = run_bass_kernel_spmd(nc, in_maps, list(range(NCORES)))

    out = np.zeros((B, E + H), dtype=np.float32)
    b3f = b3.astype(np.float32)
    for c in range(NCORES):
        att = np.asarray(res.results[c]["att"], dtype=np.float32)
        qt = np.asarray(res.results[c]["qt"], dtype=np.float32)  # [128,2,16]
        for j, seq in enumerate(core_slots[c]):
            out[seq, :E] = att[j] + b3f
            out[seq, E:E + 128] = qt[:, 0, j]
            out[seq, E + 128:] = qt[:, 1, j]
    return out
